# revision 1
# baseline (speedup 1.0000x reference)
"""Trainium2 Bass kernel for BatchSquareDiagonal.

Computes out[b] = sum_n d[b, n] * x[b, n]^2 for x, d of shape [16384, 2048]
f32, returning [16384, 1] f32. Pure data parallel across 8 NeuronCores:
core c handles batch rows [c*2048, (c+1)*2048).

Per-core pipeline (memory-bound; ~33.5 MB of input reads per core):
  - DMA row-tiles of x and d into SBUF ([128, G*2048] per tile group)
  - ScalarE (ACT): square x in SBUF
  - VectorE (DVE): tensor_tensor_reduce -> sum(x^2 * d) per partition,
    elementwise product discarded into a stride-0 broadcast dummy
  - one final DMA of the [128, 16] result block to DRAM
"""

import os
import sys

import numpy as np

for _p in ("/opt/trn_rl_repo", os.path.expanduser("~/.axon_site/_ro/trn_rl_repo")):
    if os.path.isdir(_p) and _p not in sys.path:
        sys.path.insert(0, _p)

N_CORES = 8
B, N = 16384, 2048
B_LOCAL = B // N_CORES  # 2048 rows per core
P = 128                 # SBUF partitions
G = 2                   # 128-row blocks per DMA group
N_TILES = B_LOCAL // P  # 16
N_GROUPS = N_TILES // G

_NC_CACHE = {}


def _build_nc():
    import concourse.bass as bass
    import concourse.tile as tile
    from concourse import bacc, mybir

    f32 = mybir.dt.float32
    # Bacc (not raw Bass): its compile() runs generate_event_semaphores,
    # which splits multi-sem waits (TRN2 allows 1 wait per instruction).
    nc = bacc.Bacc("TRN2", target_bir_lowering=False, debug=False)
    x = nc.declare_dram_parameter("vector", [B_LOCAL, N], f32, isOutput=False)
    d = nc.declare_dram_parameter("diag_values", [B_LOCAL, N], f32, isOutput=False)
    out = nc.declare_dram_parameter("out", [B_LOCAL, 1], f32, isOutput=True)

    # [N_TILES, P, N] views: tile j covers rows 128*j .. 128*j+127
    xv = x.ap().rearrange("(t p) n -> t p n", p=P)
    dv = d.ap().rearrange("(t p) n -> t p n", p=P)
    # out[128*j + p] == res[p, j]
    outv = out.ap().rearrange("(j p) o -> p (j o)", p=P)

    with tile.TileContext(nc) as tc:
        with (
            tc.tile_pool(name="io", bufs=3) as io_pool,
            tc.tile_pool(name="acc", bufs=1) as acc_pool,
        ):
            res = acc_pool.tile([P, N_TILES], f32)
            dummy = acc_pool.tile([P, 1], f32)

            def fused_mul_sum(sq_ap, d_ap, accum_ap):
                # one DVE pass: discard (sq*d) into a stride-0 dummy,
                # keep accum_out = sum(sq*d) per partition
                nc.vector.scalar_tensor_tensor(
                    out=dummy.broadcast_to(sq_ap.shape),
                    in0=sq_ap,
                    scalar=1.0,
                    in1=d_ap,
                    op0=mybir.AluOpType.mult,
                    op1=mybir.AluOpType.mult,
                    accum_out=accum_ap,
                )

            # Tiles 14, 15 FIRST (1 MB DMAs): under contention, ring
            # backpressure bunches trailing small loads at the end and their
            # serial square->stt chain becomes a ~15us tail. Up front their
            # compute overlaps the group loads instead.
            x14 = io_pool.tile([P, N], f32, tag="x", bufs=4)
            d14 = io_pool.tile([P, N], f32, tag="d", bufs=4)
            s14 = io_pool.tile([P, N], f32, tag="sq", bufs=3)
            nc.sync.dma_start(out=x14, in_=xv[14])
            nc.sync.dma_start(out=d14, in_=dv[14])
            nc.scalar.square(s14, x14)
            fused_mul_sum(s14[:], d14[:], res[:, 14:15])

            x15 = io_pool.tile([P, N], f32, tag="x", bufs=4)
            d15 = io_pool.tile([P, N], f32, tag="d", bufs=4)
            s15 = io_pool.tile([P, N], f32, tag="sq", bufs=3)
            nc.sync.dma_start(out=x15, in_=xv[15])
            nc.sync.dma_start(out=d15, in_=dv[15])
            nc.scalar.square(s15, x15)
            fused_mul_sum(s15[:], d15[:], res[:, 15:16])

            # Groups 0..6: [128, 2, 2048] double-row-block tiles, 2 MB DMAs.
            # Group 6 is last: its square overlaps its own d-load, so the
            # post-last-byte chain is just the two fused stts (~4.6us).
            for g in range(N_GROUPS - 1):
                xt = io_pool.tile([P, G * N], f32, tag="x", bufs=4)
                dt = io_pool.tile([P, G * N], f32, tag="d", bufs=4)
                sq = io_pool.tile([P, G * N], f32, tag="sq", bufs=3)
                xg = xv[G * g : G * g + G].transpose([1, 0, 2])
                dg = dv[G * g : G * g + G].transpose([1, 0, 2])
                nc.sync.dma_start(out=xt.rearrange("p (i n) -> p i n", i=G), in_=xg)
                nc.sync.dma_start(out=dt.rearrange("p (i n) -> p i n", i=G), in_=dg)
                nc.scalar.square(sq, xt)
                for i in range(G):
                    j = G * g + i
                    fused_mul_sum(
                        sq[:, bass.ts(i, N)], dt[:, bass.ts(i, N)], res[:, j : j + 1]
                    )
                if g == 2:
                    # cols 14,15 + 0..5 are done: store them mid-run (SWDGE;
                    # HWDGE stores flip runs into the slow mode)
                    nc.gpsimd.dma_start(out=outv[:, 14:16], in_=res[:, 14:16])
                    nc.gpsimd.dma_start(out=outv[:, :6], in_=res[:, :6])

            nc.gpsimd.dma_start(out=outv[:, 6:12], in_=res[:, 6:12])
            nc.gpsimd.dma_start(out=outv[:, 12:14], in_=res[:, 12:14])

    # Bacc.finalize() -> compile() (event-sem wait splitting, extended-ISA
    # codegen) + freeze. run_bass_via_pjrt serializes nc.m as-is, so this
    # must happen here.
    nc.finalize()
    return nc


def _build_nc_raw():
    """Raw-bass (no TileContext) pipeline with hand-rolled semaphores.

    Skips Tile's entry sem-clear butterfly and exit drain+sem-free
    barriers (~13 us combined). Standalone wait_ge instructions keep
    every op at <=1 inline wait. Semaphores are cleared at kernel end so
    the NEFF stays re-executable.
    """
    import concourse.bass as bass
    from concourse import bacc, mybir

    f32 = mybir.dt.float32
    nc = bacc.Bacc("TRN2", target_bir_lowering=False, debug=False)
    x = nc.declare_dram_parameter("vector", [B_LOCAL, N], f32, isOutput=False)
    d = nc.declare_dram_parameter("diag_values", [B_LOCAL, N], f32, isOutput=False)
    out = nc.declare_dram_parameter("out", [B_LOCAL, 1], f32, isOutput=True)

    xv = x.ap().rearrange("(t p) n -> t p n", p=P)
    dv = d.ap().rearrange("(t p) n -> t p n", p=P)
    outv = out.ap().rearrange("(j p) o -> p (j o)", p=P)

    NBX, NBD, NBS = 4, 4, 3  # slot counts: x, d, sq
    x_slots = [nc.alloc_sbuf_tensor(f"xs{i}", [P, G * N], f32) for i in range(NBX)]
    d_slots = [nc.alloc_sbuf_tensor(f"ds{i}", [P, G * N], f32) for i in range(NBD)]
    s_slots = [nc.alloc_sbuf_tensor(f"ss{i}", [P, G * N], f32) for i in range(NBS)]
    res = nc.alloc_sbuf_tensor("k_res", [P, N_TILES], f32)
    r15 = nc.alloc_sbuf_tensor("k_r15", [P, 2], f32)
    dummy = nc.alloc_sbuf_tensor("k_dummy", [P, 1], f32)

    H = N // 2
    # unit u: 0..6 = groups (rows 256u..256u+255), 7 = tile14, 8 = tile15
    # cum_stt[u]: dve_sem after unit u's stt(s)
    cum_stt = [2, 4, 6, 8, 10, 12, 14, 15, 16]
    DVE_DONE = cum_stt[-1]
    # load completion index (ld_sem = 16*m after m-th load): x_u at
    # lx[u], d-parts of unit u at ld_parts[u]
    lx = [2 * u + 1 for u in range(8)] + [17]
    ld_parts = [[2 * u + 2] for u in range(8)] + [[18, 19]]

    def x_ap(u):
        t = x_slots[u % NBX].ap()
        return t if u < 7 else t[:, :N]

    def d_ap(u):
        t = d_slots[u % NBD].ap()
        return t if u < 7 else t[:, :N]

    def s_ap(u):
        t = s_slots[u % NBS].ap()
        return t if u < 7 else t[:, :N]

    def x_src(u):
        if u < 7:
            return xv[G * u : G * u + G].transpose([1, 0, 2])
        return xv[7 + u]  # u=7 -> tile 14, u=8 -> tile 15

    ld_sem = nc.alloc_semaphore("ld")
    act_sem = nc.alloc_semaphore("act")
    dve_sem = nc.alloc_semaphore("dve")
    st_sem = nc.alloc_semaphore("st")
    # NRT does not zero semaphores at NEFF start (previous NEFFs on the
    # core leave them dirty) -> clear ours, then NRT-level barrier so no
    # engine races ahead of the clear.
    for s in (ld_sem, act_sem, dve_sem, st_sem):
        nc.gpsimd.sem_clear(s)
    nc._nrt_pseudo_barrier()

    with nc.Block() as block:

        @block.sync
        def _(sync: bass.BassEngine):
            for u in range(9):
                if u >= NBX:
                    sync.wait_ge(act_sem, u - NBX + 1)
                xd = x_ap(u)
                if u < 7:
                    xd = xd.rearrange("p (i n) -> p i n", i=G)
                sync.dma_start(out=xd, in_=x_src(u)).then_inc(ld_sem, 16)
                if u >= NBD:
                    sync.wait_ge(dve_sem, cum_stt[u - NBD])
                if u < 7:
                    dd = d_ap(u).rearrange("p (i n) -> p i n", i=G)
                    src = dv[G * u : G * u + G].transpose([1, 0, 2])
                    sync.dma_start(out=dd, in_=src).then_inc(ld_sem, 16)
                elif u == 7:
                    sync.dma_start(out=d_ap(7), in_=dv[14]).then_inc(ld_sem, 16)
                else:
                    sync.dma_start(out=d_ap(8)[:, :H], in_=dv[15][:, :H]).then_inc(
                        ld_sem, 16
                    )
                    sync.dma_start(out=d_ap(8)[:, H:], in_=dv[15][:, H:]).then_inc(
                        ld_sem, 16
                    )
            sync.wait_ge(dve_sem, DVE_DONE)
            with nc.allow_non_contiguous_dma(reason="8KB result store"):
                sync.dma_start(out=outv, in_=res.ap()).then_inc(st_sem, 16)
            sync.wait_ge(st_sem, 16)

        @block.scalar
        def _(scalar: bass.BassEngine):
            for u in range(9):
                if u >= NBS:
                    scalar.wait_ge(dve_sem, cum_stt[u - NBS])
                scalar.wait_ge(ld_sem, 16 * lx[u])
                scalar.square(s_ap(u), x_ap(u)).then_inc(act_sem, 1)

        @block.vector
        def _(vector: bass.BassEngine):
            def stt(sq_ap, dd_ap, accum_ap):
                return vector.scalar_tensor_tensor(
                    out=dummy.ap().broadcast_to(sq_ap.shape),
                    in0=sq_ap,
                    scalar=1.0,
                    in1=dd_ap,
                    op0=mybir.AluOpType.mult,
                    op1=mybir.AluOpType.mult,
                    accum_out=accum_ap,
                )

            rap = res.ap()
            for u in range(9):
                vector.wait_ge(act_sem, u + 1)
                if u < 7:
                    vector.wait_ge(ld_sem, 16 * ld_parts[u][0])
                    for i in range(G):
                        j = G * u + i
                        stt(
                            s_ap(u)[:, bass.ts(i, N)],
                            d_ap(u)[:, bass.ts(i, N)],
                            rap[:, j : j + 1],
                        ).then_inc(dve_sem, 1)
                elif u == 7:
                    vector.wait_ge(ld_sem, 16 * ld_parts[7][0])
                    stt(s_ap(7), d_ap(7), rap[:, 14:15]).then_inc(dve_sem, 1)
                else:
                    vector.wait_ge(ld_sem, 16 * ld_parts[8][1])
                    stt(s_ap(8), d_ap(8), rap[:, 15:16]).then_inc(dve_sem, 1)

    nc.finalize()
    return nc


def _get_nc():
    if "nc" not in _NC_CACHE:
        builder = _build_nc_raw if os.environ.get("RAW_KERNEL") == "1" else _build_nc
        _NC_CACHE["nc"] = builder()
    return _NC_CACHE["nc"]


def kernel(vector, diag_values):
    from concourse.bass_utils import run_bass_kernel_spmd

    vector = np.ascontiguousarray(np.asarray(vector, dtype=np.float32))
    diag_values = np.ascontiguousarray(np.asarray(diag_values, dtype=np.float32))
    assert vector.shape == (B, N) and diag_values.shape == (B, N)

    vs = vector.reshape(N_CORES, B_LOCAL, N)
    ds = diag_values.reshape(N_CORES, B_LOCAL, N)
    in_maps = [{"vector": vs[c], "diag_values": ds[c]} for c in range(N_CORES)]

    nc = _get_nc()
    res = run_bass_kernel_spmd(nc, in_maps, list(range(N_CORES)))
    return np.concatenate([res.results[c]["out"] for c in range(N_CORES)], axis=0)



# revision 3
# speedup vs baseline: 1.3544x; 1.3544x over previous
"""Trainium2 Bass kernel for BatchSquareDiagonal.

Computes out[b] = sum_n d[b, n] * x[b, n]^2 for x, d of shape [16384, 2048]
f32, returning [16384, 1] f32. Pure data parallel across 8 NeuronCores:
core c handles batch rows [c*2048, (c+1)*2048).

v2 raw-bass pipeline (memory-bound; ~33.5 MB of input reads per core):
  - INTERLEAVED row assignment: batch row b = 16*p + j lives on SBUF
    partition p, result column j. Loads are 128 x 16KB fully-contiguous
    descriptors per unit, and the final [128,16] -> [2048] result store is
    contiguous 64B per partition (vs. a 2048 x 4B scatter, whose HBM
    write-receipt tail alone cost ~9.5 us).
  - First x/d DMA issue hoisted before the NRT pseudo-barrier: their
    ld-sem incs land ~6 us after gpsimd's sem clears, so no race.
  - Last tile's d-load split in column halves + partial-sum add, so the
    post-last-byte DVE work is ~1.5 us.
  - ScalarE (ACT) squares x, VectorE (DVE) scalar_tensor_tensor does
    sum(x^2 * d) per partition into res, product discarded into a
    stride-0 dummy.
"""

import os
import sys

import numpy as np

for _p in ("/opt/trn_rl_repo", os.path.expanduser("~/.axon_site/_ro/trn_rl_repo")):
    if os.path.isdir(_p) and _p not in sys.path:
        sys.path.insert(0, _p)

N_CORES = 8
B, N = 16384, 2048
B_LOCAL = B // N_CORES  # 2048 rows per core
P = 128                 # SBUF partitions
J = B_LOCAL // P        # 16 result columns per partition (row b = 16p + j)
G = 2                   # tiles per full-size unit

_NC_CACHE = {}


def _build_nc_v2():
    """Raw-bass (no TileContext) pipeline, interleaved row layout."""
    import concourse.bass as bass
    from concourse import bacc, mybir

    f32 = mybir.dt.float32
    nc = bacc.Bacc("TRN2", target_bir_lowering=False, debug=False)
    x = nc.declare_dram_parameter("vector", [B_LOCAL, N], f32, isOutput=False)
    d = nc.declare_dram_parameter("diag_values", [B_LOCAL, N], f32, isOutput=False)
    out = nc.declare_dram_parameter("out", [B_LOCAL, 1], f32, isOutput=True)

    # row b = 16p + j  ->  xw[p, j*N + n]; per-partition bytes contiguous
    xw = x.ap().rearrange("(p j) n -> p (j n)", j=J)
    dw = d.ap().rearrange("(p j) n -> p (j n)", j=J)
    outv = out.ap().rearrange("(p j) o -> p (j o)", j=J)  # [128, 16], contiguous

    NBX, NBD, NBS = 4, 4, 3  # slot counts: x, d, sq
    W = G * N                # 4096 cols per full unit
    x_slots = [nc.alloc_sbuf_tensor(f"xs{i}", [P, W], f32) for i in range(NBX)]
    d_slots = [nc.alloc_sbuf_tensor(f"ds{i}", [P, W], f32) for i in range(NBD)]
    s_slots = [nc.alloc_sbuf_tensor(f"ss{i}", [P, W], f32) for i in range(NBS)]
    res = nc.alloc_sbuf_tensor("k_res", [P, J], f32)
    r15a = nc.alloc_sbuf_tensor("k_r15a", [P, 1], f32)
    r15b = nc.alloc_sbuf_tensor("k_r15b", [P, 1], f32)
    dummy = nc.alloc_sbuf_tensor("k_dummy", [P, 1], f32)

    # units: u=0..6 cover tiles (2u, 2u+1); u=7 -> tile 14; u=8 -> tile 15
    NU = 9
    # dve count after unit u's DVE ops (u8: stt_a, stt_b, add)
    cum_stt = [2, 4, 6, 8, 10, 12, 14, 15, 18]
    DVE_DONE = cum_stt[-1]
    # d of unit v fully read after: (its last stt that reads d)
    d_read_done = [2, 4, 6, 8, 10, 12, 14, 15, 17]
    # load order: x0,d0,x1,d1,...,x14,d14,x15,d15a,d15b  (1-based index)
    lx = [2 * u + 1 for u in range(8)] + [17]
    ld_d = [2 * u + 2 for u in range(8)] + [19]  # full d present

    H = N // 2

    def cols(u):
        return W if u < 7 else N

    def x_ap(u):
        t = x_slots[u % NBX].ap()
        return t if u < 7 else t[:, :N]

    def d_ap(u):
        t = d_slots[u % NBD].ap()
        return t if u < 7 else t[:, :N]

    def s_ap(u):
        t = s_slots[u % NBS].ap()
        return t if u < 7 else t[:, :N]

    def xsrc(u):
        if u < 7:
            return xw[:, u * W : (u + 1) * W]
        return xw[:, (7 + u) * N : (8 + u) * N]  # u=7 -> tile14, u=8 -> tile15

    def dsrc(u):
        if u < 7:
            return dw[:, u * W : (u + 1) * W]
        return dw[:, (7 + u) * N : (8 + u) * N]

    ld_sem = nc.alloc_semaphore("ld")
    act_sem = nc.alloc_semaphore("act")
    dve_sem = nc.alloc_semaphore("dve")
    st_sem = nc.alloc_semaphore("st")

    # First unit's loads issue BEFORE the sem clears + pseudo-barrier: they
    # have no waits, and their ld-sem incs land only after ~2MB streams in
    # (>=6 us after gpsimd's 15 ns range-clears). Frees ~2 us of head.
    nc.sync.dma_start(out=x_ap(0), in_=xsrc(0)).then_inc(ld_sem, 16)
    nc.sync.dma_start(out=d_ap(0), in_=dsrc(0)).then_inc(ld_sem, 16)

    # NRT does not zero semaphores at NEFF start (previous NEFFs on the
    # core leave them dirty) -> clear ours, then NRT-level barrier so no
    # consumer races ahead of the clear.
    for s in (ld_sem, act_sem, dve_sem, st_sem):
        nc.gpsimd.sem_clear(s)
    nc._nrt_pseudo_barrier()

    with nc.Block() as block:

        @block.sync
        def _(sync: bass.BassEngine):
            for u in range(1, NU):
                if u >= NBX:
                    sync.wait_ge(act_sem, u - NBX + 1)
                sync.dma_start(out=x_ap(u), in_=xsrc(u)).then_inc(ld_sem, 16)
                if u >= NBD:
                    sync.wait_ge(dve_sem, d_read_done[u - NBD])
                if u < 8:
                    sync.dma_start(out=d_ap(u), in_=dsrc(u)).then_inc(ld_sem, 16)
                else:
                    sync.dma_start(out=d_ap(8)[:, :H], in_=dsrc(8)[:, :H]).then_inc(
                        ld_sem, 16
                    )
                    sync.dma_start(out=d_ap(8)[:, H:], in_=dsrc(8)[:, H:]).then_inc(
                        ld_sem, 16
                    )
            sync.wait_ge(dve_sem, DVE_DONE)
            with nc.allow_non_contiguous_dma(reason="8KB result store"):
                sync.dma_start(out=outv, in_=res.ap()).then_inc(st_sem, 16)
            sync.wait_ge(st_sem, 16)

        @block.scalar
        def _(scalar: bass.BassEngine):
            for u in range(NU):
                if u >= NBS:
                    scalar.wait_ge(dve_sem, cum_stt[u - NBS])
                scalar.wait_ge(ld_sem, 16 * lx[u])
                scalar.square(s_ap(u), x_ap(u)).then_inc(act_sem, 1)

        @block.vector
        def _(vector: bass.BassEngine):
            def stt(sq_ap, dd_ap, accum_ap):
                return vector.scalar_tensor_tensor(
                    out=dummy.ap().broadcast_to(sq_ap.shape),
                    in0=sq_ap,
                    scalar=1.0,
                    in1=dd_ap,
                    op0=mybir.AluOpType.mult,
                    op1=mybir.AluOpType.mult,
                    accum_out=accum_ap,
                )

            rap = res.ap()
            for u in range(7):
                vector.wait_ge(act_sem, u + 1)
                vector.wait_ge(ld_sem, 16 * ld_d[u])
                for g in range(G):
                    j = G * u + g
                    stt(
                        s_ap(u)[:, bass.ts(g, N)],
                        d_ap(u)[:, bass.ts(g, N)],
                        rap[:, j : j + 1],
                    ).then_inc(dve_sem, 1)
            # tile 14
            vector.wait_ge(act_sem, 8)
            vector.wait_ge(ld_sem, 16 * 16)
            stt(s_ap(7), d_ap(7), rap[:, 14:15]).then_inc(dve_sem, 1)
            # tile 15: d in column halves, partial sums added at the end
            vector.wait_ge(act_sem, 9)
            vector.wait_ge(ld_sem, 16 * 18)
            stt(s_ap(8)[:, :H], d_ap(8)[:, :H], r15a.ap()).then_inc(dve_sem, 1)
            vector.wait_ge(ld_sem, 16 * 19)
            stt(s_ap(8)[:, H:], d_ap(8)[:, H:], r15b.ap()).then_inc(dve_sem, 1)
            # RAW hazard: the accumulator flush writing r15b retires late;
            # drain the DVE pipe before reading it back.
            vector.drain()
            vector.scalar_tensor_tensor(
                out=rap[:, 15:16],
                in0=r15a.ap(),
                scalar=0.0,
                in1=r15b.ap(),
                op0=mybir.AluOpType.add,
                op1=mybir.AluOpType.add,
            ).then_inc(dve_sem, 1)

    nc.finalize()
    return nc


def _build_nc_tile():
    """Tile-based fallback (previous session's kernel, proven correct)."""
    import concourse.bass as bass
    import concourse.tile as tile
    from concourse import bacc, mybir

    f32 = mybir.dt.float32
    nc = bacc.Bacc("TRN2", target_bir_lowering=False, debug=False)
    x = nc.declare_dram_parameter("vector", [B_LOCAL, N], f32, isOutput=False)
    d = nc.declare_dram_parameter("diag_values", [B_LOCAL, N], f32, isOutput=False)
    out = nc.declare_dram_parameter("out", [B_LOCAL, 1], f32, isOutput=True)

    N_TILES = B_LOCAL // P  # 16
    N_GROUPS = N_TILES // G

    xv = x.ap().rearrange("(t p) n -> t p n", p=P)
    dv = d.ap().rearrange("(t p) n -> t p n", p=P)
    outv = out.ap().rearrange("(j p) o -> p (j o)", p=P)

    with tile.TileContext(nc) as tc:
        with (
            tc.tile_pool(name="io", bufs=3) as io_pool,
            tc.tile_pool(name="acc", bufs=1) as acc_pool,
        ):
            res = acc_pool.tile([P, N_TILES], f32)
            dummy = acc_pool.tile([P, 1], f32)

            def fused_mul_sum(sq_ap, d_ap, accum_ap):
                nc.vector.scalar_tensor_tensor(
                    out=dummy.broadcast_to(sq_ap.shape),
                    in0=sq_ap,
                    scalar=1.0,
                    in1=d_ap,
                    op0=mybir.AluOpType.mult,
                    op1=mybir.AluOpType.mult,
                    accum_out=accum_ap,
                )

            x14 = io_pool.tile([P, N], f32, tag="x", bufs=4)
            d14 = io_pool.tile([P, N], f32, tag="d", bufs=4)
            s14 = io_pool.tile([P, N], f32, tag="sq", bufs=3)
            nc.sync.dma_start(out=x14, in_=xv[14])
            nc.sync.dma_start(out=d14, in_=dv[14])
            nc.scalar.square(s14, x14)
            fused_mul_sum(s14[:], d14[:], res[:, 14:15])

            x15 = io_pool.tile([P, N], f32, tag="x", bufs=4)
            d15 = io_pool.tile([P, N], f32, tag="d", bufs=4)
            s15 = io_pool.tile([P, N], f32, tag="sq", bufs=3)
            nc.sync.dma_start(out=x15, in_=xv[15])
            nc.sync.dma_start(out=d15, in_=dv[15])
            nc.scalar.square(s15, x15)
            fused_mul_sum(s15[:], d15[:], res[:, 15:16])

            for g in range(N_GROUPS - 1):
                xt = io_pool.tile([P, G * N], f32, tag="x", bufs=4)
                dt = io_pool.tile([P, G * N], f32, tag="d", bufs=4)
                sq = io_pool.tile([P, G * N], f32, tag="sq", bufs=3)
                xg = xv[G * g : G * g + G].transpose([1, 0, 2])
                dg = dv[G * g : G * g + G].transpose([1, 0, 2])
                nc.sync.dma_start(out=xt.rearrange("p (i n) -> p i n", i=G), in_=xg)
                nc.sync.dma_start(out=dt.rearrange("p (i n) -> p i n", i=G), in_=dg)
                nc.scalar.square(sq, xt)
                for i in range(G):
                    j = G * g + i
                    fused_mul_sum(
                        sq[:, bass.ts(i, N)], dt[:, bass.ts(i, N)], res[:, j : j + 1]
                    )
                if g == 2:
                    nc.gpsimd.dma_start(out=outv[:, 14:16], in_=res[:, 14:16])
                    nc.gpsimd.dma_start(out=outv[:, :6], in_=res[:, :6])

            nc.gpsimd.dma_start(out=outv[:, 6:12], in_=res[:, 6:12])
            nc.gpsimd.dma_start(out=outv[:, 12:14], in_=res[:, 12:14])

    nc.finalize()
    return nc


def _get_nc():
    if "nc" not in _NC_CACHE:
        builder = (
            _build_nc_tile if os.environ.get("TILE_KERNEL") == "1" else _build_nc_v2
        )
        _NC_CACHE["nc"] = builder()
    return _NC_CACHE["nc"]


def kernel(vector, diag_values):
    from concourse.bass_utils import run_bass_kernel_spmd

    vector = np.ascontiguousarray(np.asarray(vector, dtype=np.float32))
    diag_values = np.ascontiguousarray(np.asarray(diag_values, dtype=np.float32))
    assert vector.shape == (B, N) and diag_values.shape == (B, N)

    vs = vector.reshape(N_CORES, B_LOCAL, N)
    ds = diag_values.reshape(N_CORES, B_LOCAL, N)
    in_maps = [{"vector": vs[c], "diag_values": ds[c]} for c in range(N_CORES)]

    nc = _get_nc()
    res = run_bass_kernel_spmd(nc, in_maps, list(range(N_CORES)))
    return np.concatenate([res.results[c]["out"] for c in range(N_CORES)], axis=0)
